# revision 13
# baseline (speedup 1.0000x reference)
"""Trainium2 Bass kernel for blended-expert MLP (moe_routing).

Model: for each of 3 layers, w_l = sum_e c_e * W[l,e]  (E=8 experts),
x = act(x @ w_l.T + B_l), act = ELU for layers 0,1, none for layer 2.

Strategy (8 NeuronCores):
- Data-parallel over the 4096-row batch (512 rows/core).
- The expert blend is sharded over the contraction (input) dim: core k blends
  a 256-row slice of w_l.T on the Vector engine (bf16 accumulate, 2x DVE),
  writing straight into an SBUF weight buffer.
- The blended slices are exchanged SBUF->SBUF with remote_dma_broadcast
  (single-destination relative sends at each physical-XOR distance j into a
  compile-time slot j), so the full blended w_l.T lives in SBUF on every core
  with zero DRAM round-trip and no CC collective on the critical path.
- All layout permutations induced by the XOR addressing are absorbed into
  host-side input prep (feature blocks of x / weight columns / biases are
  XOR-permuted per core) plus compile-time column indexing of the stationary
  matmul operand. A tiny probe kernel resolves the logical->physical core
  mapping once; if the topology is not XOR-consistent we fall back to the
  previous CC-AllGather implementation (kept below).
- Matmuls run in bf16 (fp32 PSUM accumulation); ELU is fused DVE/ACT ops.
"""

import numpy as np
from contextlib import ExitStack

import concourse.mybir as mybir
import concourse.tile as tile
from concourse import bacc
from concourse.bass_utils import run_bass_kernel_spmd

N_CORES = 8
L = 3          # layers
E = 8          # experts
D = 2048       # feature dim
BATCH = 4096
BS = BATCH // N_CORES   # 512 batch rows per core
IS = D // N_CORES       # 256 contraction rows blended per core
KT = D // 128           # 16 k-tiles
MT = D // 128           # 16 m-tiles (output feature tiles)
HALF_M = MT // 2        # 8 psum banks per half

f32 = mybir.dt.float32
f32r = mybir.dt.float32r
bf16 = mybir.dt.bfloat16

_cache: dict = {}


def _wge(inst, sem, val):
    """Attach a wait to an instruction; bacc's generate_event_semaphores
    splits multi-wait instructions into event-semaphore preludes."""
    inst.wait_op(sem, val, "sem-ge", check=False)
    return inst


# ---------------------------------------------------------------------------
# Topology probe: each core sends its logical rank to the core at physical
# XOR distance j, landing in column j of an id table. The host derives the
# logical->physical-XOR labeling phi from the result.
# ---------------------------------------------------------------------------

def _build_probe():
    nc = bacc.Bacc("TRN2", target_bir_lowering=False, debug=False,
                   num_devices=N_CORES)
    myid = nc.dram_tensor("myid", [128, 1], f32, kind="ExternalInput")
    idout = nc.dram_tensor("idout", [128, 8], f32, kind="ExternalOutput")

    psem = nc.alloc_semaphore("probe_arrive")
    plocal = nc.alloc_semaphore("probe_local")

    with tile.TileContext(nc) as tc:
        with tc.tile_pool(name="p", bufs=1) as pool:
            idin = pool.tile([128, 1], f32)
            idtab = pool.tile([128, 8], f32)
            nc.sync.dma_start(idin[:], myid[:])
            # own rank in column 0
            nc.sync.dma_start(idtab[:, 0:1], myid[:])
            for j in range(1, 8):
                rd: list = [None] * 8
                rd[j] = (0, j)
                nc.gpsimd.remote_dma_broadcast(
                    idtab[:, j:j + 1], idin[:, 0:1], psem, plocal, rdests=rd
                )
            # Tile orders: load -> prep (shadow dep) -> trigger (pool sem)
            nc.gpsimd.trigger_dma(count=None)
            out = nc.sync.dma_start(idout[:], idtab[:])
    # attach remote-arrival wait after Tile scheduling (its single-core
    # scheduling sim cannot see remote sem increments and would deadlock)
    _wge(out, psem, 14)  # 7 senders x 2 lanes
    nc.finalize()
    return nc


def _resolve_phi():
    """Returns phi (list: logical rank -> physical-xor label) or None."""
    if "phi" in _cache:
        return _cache["phi"]
    phi = None
    try:
        nc = _build_probe()
        in_maps = [{"myid": np.full((128, 1), c, dtype=np.float32)}
                   for c in range(N_CORES)]
        res = run_bass_kernel_spmd(nc, in_maps, core_ids=list(range(N_CORES)))
        ids = [[int(round(float(res.results[c]["idout"][0, j])))
                for j in range(8)] for c in range(N_CORES)]
        cand = [None] * 8
        cand[ids[0][0]] = 0
        for j in range(1, 8):
            s = ids[0][j]
            if not (0 <= s < 8) or cand[s] is not None:
                raise ValueError(f"bad probe row0: {ids[0]}")
            cand[s] = j
        if any(v is None for v in cand):
            raise ValueError(f"probe not a permutation: {ids[0]}")
        for c in range(8):
            if ids[c][0] != c:
                raise ValueError(f"own-id mismatch: {ids[c]}")
            for j in range(1, 8):
                if cand[ids[c][j]] != cand[c] ^ j:
                    raise ValueError(f"non-XOR topology: {ids}")
        phi = cand
    except Exception as e:  # noqa: BLE001 - fall back to CC kernel
        import sys
        print(f"[kernel] topology probe failed ({e!r}); "
              f"falling back to CC AllGather path", file=sys.stderr)
        phi = None
    _cache["phi"] = phi
    return phi


# ---------------------------------------------------------------------------
# Main kernel: remote-DMA weight exchange version.
# ---------------------------------------------------------------------------

def _build_main():
    nc = bacc.Bacc("TRN2", target_bir_lowering=False, debug=False,
                   num_devices=N_CORES)
    # Per-core inputs (pre-sliced/permuted by host, see make_in_maps):
    # WtT: (L, E, IS, D) = this core's in-row slice of w.T per expert, with
    #      out-columns XOR-permuted into this core's position order, bf16.
    WtT = nc.dram_tensor("WtT", [L, E, IS, D], bf16, kind="ExternalInput")
    # xT: (D, BS) = batch shard transposed, feature blocks position-ordered.
    xT = nc.dram_tensor("xT", [D, BS], bf16, kind="ExternalInput")
    # cb: (128, E) = weight_blend broadcast across partitions
    cb = nc.dram_tensor("cb", [128, E], f32, kind="ExternalInput")
    # Bp: (128, L*MT) = biases; col l*MT+m holds the position-m bias tile.
    Bp = nc.dram_tensor("Bp", [128, L * MT], f32, kind="ExternalInput")
    # Output: yT (D, BS), feature blocks position-ordered.
    yT = nc.dram_tensor("yT", [D, BS], f32, kind="ExternalOutput")

    # Semaphores
    sendloc = nc.alloc_semaphore("sendloc")
    slotsem = [[[nc.alloc_semaphore(f"slot{l}_{j}_{h}") for h in range(2)]
                for j in range(8)] for l in range(L)]
    mm0done = nc.alloc_semaphore("mm0done")
    creditsem = nc.alloc_semaphore("credit")
    credloc = nc.alloc_semaphore("credloc")

    cum_prep = [0]
    post_waits = []  # (inst, sem, val) attached after Tile scheduling

    with ExitStack() as stack:
        # Raw SBUF weight buffers: wbuf[b][j] holds slot j ([2 row-halves x
        # 2048 cols] of the blended w_l.T slice from physical distance j).
        # b=0 serves layers 0 and 2, b=1 serves layer 1.
        wbuf = [[stack.enter_context(
                    nc.sbuf_tensor(f"wb{b}_{j}", [128, 2 * D], bf16))
                 for j in range(8)] for b in range(2)]

        tc = stack.enter_context(tile.TileContext(nc))
        cpool = stack.enter_context(tc.tile_pool(name="const", bufs=1))
        apool = stack.enter_context(tc.tile_pool(name="acts", bufs=1))
        epool = stack.enter_context(tc.tile_pool(name="exp", bufs=3))
        xpool = stack.enter_context(tc.tile_pool(name="epi", bufs=2))
        ppool = stack.enter_context(tc.tile_pool(name="psum", bufs=1,
                                                 space="PSUM"))

        cbt = cpool.tile([128, E], f32)
        bt = cpool.tile([128, L * MT], f32)
        nc.sync.dma_start(cbt[:], cb[:])
        nc.sync.dma_start(bt[:], Bp[:])


        # Activations: two sets of KT tiles [128, BS], ping-pong.
        acts = [
            [apool.tile([128, BS], bf16, name=f"act{s}_{k}")
             for k in range(KT)]
            for s in range(2)
        ]
        for k in range(KT):
            nc.sync.dma_start(acts[0][k][:], xT[k * 128:(k + 1) * 128, :])

        def emit_blend(l, b):
            dst = wbuf[b][0]
            first = None
            op = None
            for e in range(E):
                et = epool.tile([128, 2 * D], bf16, name=f"exp{l}_{e}",
                                tag="exp")
                nc.scalar.dma_start(et[:, 0:D], WtT[l, e, 0:128, :])
                nc.scalar.dma_start(et[:, D:2 * D], WtT[l, e, 128:256, :])
                if e == 0:
                    op = nc.vector.tensor_scalar_mul(
                        dst[:], et[:], cbt[:, 0:1])
                    first = op
                else:
                    op = nc.vector.scalar_tensor_tensor(
                        dst[:], et[:], cbt[:, e:e + 1], dst[:],
                        mybir.AluOpType.mult, mybir.AluOpType.add,
                    )
            return first

        def emit_sends(l, b):
            # two sequential chunks per destination on the same lane pair
            # (slot j); concurrent dual-lane transfers to one destination
            # trip a multi-ms ucode pathology.
            for h in range(2):
                for j in range(1, 8):
                    rd: list = [None] * 8
                    rd[j] = (0, j)
                    nc.gpsimd.remote_dma_broadcast(
                        wbuf[b][j][:, h * D:(h + 1) * D],
                        wbuf[b][0][:, h * D:(h + 1) * D],
                        slotsem[l][j][h], sendloc, rdests=rd,
                    )
            trig = nc.gpsimd.trigger_dma(count=None)
            if l == 2:
                # every core finished reading wbuf[0] (its mm0) before the
                # layer-2 slices overwrite it remotely.
                post_waits.append((trig, creditsem, 16))
            return trig

        def emit_mm(l, b):
            src = acts[l % 2]
            dst = acts[(l + 1) % 2]
            waited = set()
            last = None
            for half in range(2):
                psums = [
                    ppool.tile([128, BS], f32,
                               name=f"ps{l}_{half}_{m}", tag=f"bank{m}")
                    for m in range(HALF_M)
                ]
                for kt in range(KT):
                    j, h = kt // 2, kt % 2
                    for m8 in range(HALF_M):
                        m = half * HALF_M + m8
                        col = h * D + ((m // 2) ^ j) * 256 + (m % 2) * 128
                        mm = nc.tensor.matmul(
                            psums[m8][:],
                            wbuf[b][j][:, col:col + 128],
                            src[kt][:],
                            start=(kt == 0),
                            stop=(kt == KT - 1),
                        )
                        if j > 0 and (j, h) not in waited:
                            post_waits.append((mm, slotsem[l][j][h], 2))
                            waited.add((j, h))
                        last = mm
                # epilogue: bias (+ ELU), write next-layer acts
                for m8 in range(HALF_M):
                    gm = half * HALF_M + m8
                    bias = bt[:, l * MT + gm: l * MT + gm + 1]
                    ps = psums[m8]
                    if l < L - 1:
                        tt = xpool.tile([128, BS], f32,
                                        name=f"t{l}_{gm}", tag="tmin")
                        zt = xpool.tile([128, BS], f32,
                                        name=f"z{l}_{gm}", tag="zbias")
                        ut = xpool.tile([128, BS], f32,
                                        name=f"u{l}_{gm}", tag="uexp")
                        # t = min(psum + bias, 0) on DVE
                        nc.vector.tensor_scalar(
                            tt[:], ps[:], bias, 0.0,
                            mybir.AluOpType.add, mybir.AluOpType.min,
                        )
                        # z = psum + bias on ACT
                        nc.scalar.activation(
                            zt[:], ps[:],
                            mybir.ActivationFunctionType.Identity,
                            bias=bias,
                        )
                        # u = exp(t) on ACT
                        nc.scalar.activation(
                            ut[:], tt[:], mybir.ActivationFunctionType.Exp
                        )
                        # act_next = max(u - 1, z) on DVE, in matmul dtype
                        nc.vector.scalar_tensor_tensor(
                            dst[gm][:], ut[:], 1.0, zt[:],
                            mybir.AluOpType.subtract, mybir.AluOpType.max,
                        )
                    else:
                        ot = xpool.tile([128, BS], f32,
                                        name=f"o{gm}", tag="outt")
                        nc.scalar.activation(
                            ot[:], ps[:],
                            mybir.ActivationFunctionType.Identity,
                            bias=bias,
                        )
                        nc.gpsimd.dma_start(
                            yT[gm * 128:(gm + 1) * 128, :], ot[:]
                        )
            return last

        emit_blend(0, 0)
        emit_sends(0, 0)
        emit_blend(1, 1)
        emit_sends(1, 1)

        emit_mm(0, 0)
        # scheduler fence + DVE-side completion signal: the last layer-0
        # epilogue op only issues after the final matmul fully completed
        # (PSUM wait), so this inc proves every wbuf[0] read retired.
        tc.no_sync_barrier()
        nc.vector.sem_inc(mm0done, 1)

        # credit broadcast: tells every core (incl. self) that this core's
        # mm0 stopped reading wbuf[0].
        nc.gpsimd.remote_sem_update_broadcast(
            creditsem, credloc, rdests=[(0, k) for k in range(8)]
        )
        ctrig = nc.gpsimd.trigger_dma(count=None)
        _wge(ctrig, mm0done, 1)

        b2first = emit_blend(2, 0)
        # blend 2 overwrites wbuf[0][0]: wait for mm0's reads and for the
        # layer-0/1 sends (28 x 16 on sendloc) to have left the buffer.
        _wge(b2first, mm0done, 1)
        post_waits.append((b2first, sendloc, 28 * 16))
        emit_sends(2, 0)

        emit_mm(1, 1)
        emit_mm(2, 0)

        # TileContext exits here (stack unwinds in LIFO order), then wbuf.
        stack.pop_all().close()
    for inst, sem, val in post_waits:
        _wge(inst, sem, val)
    nc.finalize()
    return nc


def _get_main():
    if "main" not in _cache:
        _cache["main"] = _build_main()
    return _cache["main"]


def make_in_maps(weight_blend, x, W, B, phi):
    weight_blend = np.asarray(weight_blend, dtype=np.float32)
    x = np.asarray(x, dtype=np.float32)
    W = np.asarray(W, dtype=np.float32)
    B = np.asarray(B, dtype=np.float32)

    import ml_dtypes
    cbv = np.ascontiguousarray(
        np.broadcast_to(weight_blend[None, :], (128, E)))
    in_maps = []
    for c in range(N_CORES):
        f = phi[c]
        perm = [f ^ q for q in range(8)]  # position block q -> abs block
        # WtT: in-rows = abs block f of w.T (i.e. W[..., f*IS:(f+1)*IS]),
        # out-columns position-ordered.
        Wc = W[:, :, :, f * IS:(f + 1) * IS]           # [L, E, D, IS]
        WT = Wc.transpose(0, 1, 3, 2)                   # [L, E, IS, D]
        WTp = WT.reshape(L, E, IS, 8, IS)[:, :, :, perm, :].reshape(
            L, E, IS, D)
        WtTc = np.ascontiguousarray(WTp).astype(ml_dtypes.bfloat16)
        # xT: batch shard (by logical rank), feature blocks position-ordered
        xTc = x[c * BS:(c + 1) * BS, :].T               # [D, BS]
        xTp = np.ascontiguousarray(
            xTc.reshape(8, IS, BS)[perm].reshape(D, BS)
        ).astype(ml_dtypes.bfloat16)
        # Bp[p, l*MT+m] = B[l, (f^(m//2))*256 + (m%2)*128 + p]
        Babs = B.reshape(L, 8, 2, 128)                  # [L, blk, sub, p]
        Bp = np.ascontiguousarray(
            Babs[:, perm, :, :].transpose(3, 0, 1, 2).reshape(128, L * MT))
        in_maps.append({"WtT": WtTc, "xT": xTp, "cb": cbv, "Bp": Bp})
    return in_maps


def _run_main(weight_blend, x, W, B, phi):
    in_maps = make_in_maps(weight_blend, x, W, B, phi)
    nc = _get_main()
    res = run_bass_kernel_spmd(nc, in_maps, core_ids=list(range(N_CORES)))
    out = np.empty((BATCH, D), dtype=np.float32)
    for c in range(N_CORES):
        f = phi[c]
        perm = [f ^ q for q in range(8)]
        y = np.asarray(res.results[c]["yT"]).reshape(8, IS, BS)
        out[c * BS:(c + 1) * BS, :] = y[perm].reshape(D, BS).T
    return out


# ---------------------------------------------------------------------------
# Fallback: previous CC-AllGather implementation (verbatim).
# ---------------------------------------------------------------------------

def _build_cc():
    nc = bacc.Bacc("TRN2", target_bir_lowering=False, debug=False,
                   num_devices=N_CORES)
    WtT = nc.dram_tensor("WtT", [L, E, IS, D], bf16, kind="ExternalInput")
    xT = nc.dram_tensor("xT", [D, BS], bf16, kind="ExternalInput")
    cb = nc.dram_tensor("cb", [128, E], f32, kind="ExternalInput")
    Bp = nc.dram_tensor("Bp", [128, L * MT], f32, kind="ExternalInput")
    yT = nc.dram_tensor("yT", [D, BS], f32, kind="ExternalOutput")

    with tile.TileContext(nc) as tc:
        with (
            tc.tile_pool(name="const", bufs=1) as cpool,
            tc.tile_pool(name="acts", bufs=1) as apool,
            tc.tile_pool(name="blend", bufs=1) as bpool,
            tc.tile_pool(name="exp", bufs=8) as epool,
            tc.tile_pool(name="wst", bufs=12) as wpool,
            tc.tile_pool(name="epi", bufs=6) as xpool,
            tc.tile_pool(name="psum", bufs=1, space="PSUM") as ppool,
            tc.tile_pool(name="dram", bufs=1, space="DRAM") as dram,
        ):
            cbt = cpool.tile([128, E], f32)
            bt = cpool.tile([128, L * MT], f32)
            nc.sync.dma_start(cbt[:], cb[:])
            nc.sync.dma_start(bt[:], Bp[:])

            wuin = dram.tile([1, E], f32, name="wuin")
            wuout = dram.tile([N_CORES, E], f32, addr_space="Shared",
                              name="wuout")
            nc.sync.dma_start(wuin[:], cb[:1, :])
            nc.gpsimd.collective_compute(
                "AllGather", mybir.AluOpType.bypass,
                ins=[wuin.opt()], outs=[wuout.opt()],
                replica_groups=[list(range(N_CORES))],
            )

            acts = [
                [apool.tile([128, BS], bf16, name=f"act{s}_{k}")
                 for k in range(KT)]
                for s in range(2)
            ]
            agins = [dram.tile([IS, D], bf16, name=f"agin{l}")
                     for l in range(L)]
            agouts = [
                dram.tile([D, D], bf16, addr_space="Shared", name=f"agout{l}")
                for l in range(L)
            ]

            def emit_blend(l):
                for h in range(IS // 128):
                    acc = bpool.tile([128, D], f32, name=f"acc{l}_{h}",
                                     tag=f"acc{h}", bufs=2)
                    accq = bpool.tile([128, D], bf16, name=f"accq{l}_{h}",
                                      tag=f"accq{h}", bufs=2)
                    for e in range(E):
                        et = epool.tile([128, D], bf16, name=f"exp{l}_{h}_{e}",
                                        tag="exp")
                        nc.scalar.dma_start(
                            et[:], WtT[l, e, h * 128:(h + 1) * 128, :]
                        )
                        if e == 0:
                            nc.vector.tensor_scalar_mul(
                                acc[:], et[:], cbt[:, 0:1]
                            )
                        elif e < E - 1:
                            nc.vector.scalar_tensor_tensor(
                                acc[:], et[:], cbt[:, e:e + 1], acc[:],
                                mybir.AluOpType.mult, mybir.AluOpType.add,
                            )
                        else:
                            nc.vector.scalar_tensor_tensor(
                                accq[:], et[:], cbt[:, e:e + 1], acc[:],
                                mybir.AluOpType.mult, mybir.AluOpType.add,
                            )
                    nc.gpsimd.dma_start(
                        agins[l][h * 128:(h + 1) * 128, :], accq[:]
                    )
                nc.gpsimd.collective_compute(
                    "AllGather", mybir.AluOpType.bypass,
                    ins=[agins[l].opt()], outs=[agouts[l].opt()],
                    replica_groups=[list(range(N_CORES))],
                )

            emit_blend(0)
            emit_blend(1)

            for k in range(KT):
                nc.sync.dma_start(acts[0][k][:], xT[k * 128:(k + 1) * 128, :])

            for l in range(L):
                if l == 1:
                    emit_blend(2)
                src = acts[l % 2]
                dst = acts[(l + 1) % 2]
                for half in range(2):
                    psums = [
                        ppool.tile([128, BS], f32,
                                   name=f"ps{l}_{half}_{m}", tag=f"bank{m}")
                        for m in range(HALF_M)
                    ]
                    for k in range(KT):
                        ws = wpool.tile([128, HALF_M * 128], bf16,
                                        name=f"ws{l}_{half}_{k}", tag="ws")
                        nc.sync.dma_start(
                            ws[:],
                            agouts[l][
                                k * 128:(k + 1) * 128,
                                half * HALF_M * 128:(half + 1) * HALF_M * 128,
                            ],
                        )
                        for m in range(HALF_M):
                            nc.tensor.matmul(
                                psums[m][:],
                                ws[:, m * 128:(m + 1) * 128],
                                src[k][:],
                                start=(k == 0),
                                stop=(k == KT - 1),
                            )
                    for m in range(HALF_M):
                        gm = half * HALF_M + m
                        bias = bt[:, l * MT + gm: l * MT + gm + 1]
                        ps = psums[m]
                        if l < L - 1:
                            tt = xpool.tile([128, BS], f32,
                                            name=f"t{l}_{gm}", tag="tmin")
                            zt = xpool.tile([128, BS], f32,
                                            name=f"z{l}_{gm}", tag="zbias")
                            ut = xpool.tile([128, BS], f32,
                                            name=f"u{l}_{gm}", tag="uexp")
                            nc.vector.tensor_scalar(
                                tt[:], ps[:], bias, 0.0,
                                mybir.AluOpType.add, mybir.AluOpType.min,
                            )
                            nc.scalar.activation(
                                zt[:], ps[:],
                                mybir.ActivationFunctionType.Identity,
                                bias=bias,
                            )
                            nc.scalar.activation(
                                ut[:], tt[:], mybir.ActivationFunctionType.Exp
                            )
                            nc.vector.scalar_tensor_tensor(
                                dst[gm][:], ut[:], 1.0, zt[:],
                                mybir.AluOpType.subtract, mybir.AluOpType.max,
                            )
                        else:
                            ot = xpool.tile([128, BS], f32,
                                            name=f"o{gm}", tag="outt")
                            nc.scalar.activation(
                                ot[:], ps[:],
                                mybir.ActivationFunctionType.Identity,
                                bias=bias,
                            )
                            nc.gpsimd.dma_start(
                                yT[gm * 128:(gm + 1) * 128, :], ot[:]
                            )
    nc.finalize()
    return nc


def _get_cc():
    if "cc" not in _cache:
        _cache["cc"] = _build_cc()
    return _cache["cc"]


def make_in_maps_cc(weight_blend, x, W, B):
    weight_blend = np.asarray(weight_blend, dtype=np.float32)
    x = np.asarray(x, dtype=np.float32)
    W = np.asarray(W, dtype=np.float32)
    B = np.asarray(B, dtype=np.float32)

    cb = np.ascontiguousarray(np.broadcast_to(weight_blend[None, :], (128, E)))
    Bp = np.ascontiguousarray(
        B.reshape(L, MT, 128).transpose(2, 0, 1).reshape(128, L * MT)
    )
    import ml_dtypes
    in_maps = []
    for k in range(N_CORES):
        WtT = np.ascontiguousarray(
            W[:, :, :, k * IS:(k + 1) * IS].transpose(0, 1, 3, 2)
        ).astype(ml_dtypes.bfloat16)
        xTk = np.ascontiguousarray(
            x[k * BS:(k + 1) * BS, :].T).astype(ml_dtypes.bfloat16)
        in_maps.append({"WtT": WtT, "xT": xTk, "cb": cb, "Bp": Bp})
    return in_maps


def _run_cc(weight_blend, x, W, B):
    in_maps = make_in_maps_cc(weight_blend, x, W, B)
    nc = _get_cc()
    res = run_bass_kernel_spmd(nc, in_maps, core_ids=list(range(N_CORES)))
    out = np.empty((BATCH, D), dtype=np.float32)
    for k in range(N_CORES):
        out[k * BS:(k + 1) * BS, :] = res.results[k]["yT"].T
    return out


# ---------------------------------------------------------------------------


def kernel(weight_blend, x, W, B) -> np.ndarray:
    phi = _resolve_phi()
    last_err = None
    for attempt in range(3):
        try:
            if phi is not None:
                out = _run_main(weight_blend, x, W, B, phi)
            else:
                out = _run_cc(weight_blend, x, W, B)
            if np.isfinite(out).all():
                return out
            last_err = RuntimeError("non-finite kernel output")
        except Exception as e:  # transient NRT/device wedge: retry
            last_err = e
        import time as _time
        _time.sleep(10 * (attempt + 1))
    raise last_err


# revision 15
# speedup vs baseline: 59.6689x; 59.6689x over previous
"""Trainium2 Bass kernel for blended-expert MLP (moe_routing).

Model: for each of 3 layers, w_l = sum_e c_e * W[l,e]  (E=8 experts),
x = act(x @ w_l.T + B_l), act = ELU for layers 0,1, none for layer 2.

Strategy (8 NeuronCores):
- Data-parallel over the 4096-row batch (512 rows/core).
- The expert blend is sharded over the contraction (input) dim: core k blends
  i-slice k (256 rows of w_l.T) on the Vector engine; 8-core AllGathers
  assemble the full transposed blended weight w_l.T in DRAM, which the
  matmul phase streams as stationary operands.
- The AllGathers are CHUNKED (layer 0 in four 64-row pieces, layers 1-2 in
  two 128-row pieces) and a tiny warmup AllGather is issued first: the
  ~65us first-collective staging cost and the cross-core launch skew are
  absorbed while the expert loads + blend run, and the first real chunk
  completes ~45us earlier than a monolithic AllGather would - the matmuls
  start as soon as chunk 0 lands and are paced by the remaining chunks.
- Matmuls run in bf16 (fp32 PSUM accumulation); expert weights and x are
  uploaded as bf16, halving the dominant HBM streams (rel err ~5e-3).
  Activations stay SBUF-resident between layers in [feature, batch]
  orientation; ELU is fused DVE/ACT ops: max(exp(min(z,0))-1, z).
- Host side only reshapes/transposes/slices for sharding and assembles the
  output; all FLOPs (blend, matmul, bias, ELU) run on device.
"""

import numpy as np

import concourse.mybir as mybir
import concourse.tile as tile
from concourse import bacc
from concourse.bass_utils import run_bass_kernel_spmd

N_CORES = 8
L = 3          # layers
E = 8          # experts
D = 2048       # feature dim
BATCH = 4096
BS = BATCH // N_CORES   # 512 batch rows per core
IS = D // N_CORES       # 256 contraction rows blended per core
KT = D // 128           # 16 k-tiles
MT = D // 128           # 16 m-tiles (output feature tiles)
HALF_M = MT // 2        # 8 psum banks per half

f32 = mybir.dt.float32
bf16 = mybir.dt.bfloat16

# AllGather chunks per layer: layer 0 finest (it gates the first matmul).
CHUNKS = [4, 2, 2]

_cache: dict = {}


def _build():
    nc = bacc.Bacc("TRN2", target_bir_lowering=False, debug=False,
                   num_devices=N_CORES)
    # WtT: (L, E, IS, D) = this core's i-slice of W transposed to [in, out]
    WtT = nc.dram_tensor("WtT", [L, E, IS, D], bf16, kind="ExternalInput")
    # xT: (D, BS) = this core's batch shard, transposed
    xT = nc.dram_tensor("xT", [D, BS], bf16, kind="ExternalInput")
    cb = nc.dram_tensor("cb", [128, E], f32, kind="ExternalInput")
    Bp = nc.dram_tensor("Bp", [128, L * MT], f32, kind="ExternalInput")
    yT = nc.dram_tensor("yT", [D, BS], f32, kind="ExternalOutput")

    with tile.TileContext(nc) as tc:
        with (
            tc.tile_pool(name="const", bufs=1) as cpool,
            tc.tile_pool(name="acts", bufs=1) as apool,
            tc.tile_pool(name="blend", bufs=1) as bpool,
            tc.tile_pool(name="exp", bufs=8) as epool,
            tc.tile_pool(name="wst", bufs=12) as wpool,
            tc.tile_pool(name="epi", bufs=6) as xpool,
            tc.tile_pool(name="psum", bufs=1, space="PSUM") as ppool,
            tc.tile_pool(name="dram", bufs=1, space="DRAM") as dram,
        ):
            cbt = cpool.tile([128, E], f32)
            bt = cpool.tile([128, L * MT], f32)
            nc.sync.dma_start(cbt[:], cb[:])
            nc.sync.dma_start(bt[:], Bp[:])

            # Warmup AllGather: absorbs cross-core launch skew and the
            # first-collective staging cost while the bulk loads run.
            wuin = dram.tile([1, E], f32, name="wuin")
            wuout = dram.tile([N_CORES, E], f32, addr_space="Shared",
                              name="wuout")
            nc.sync.dma_start(wuin[:], cb[:1, :])
            nc.gpsimd.collective_compute(
                "AllGather", mybir.AluOpType.bypass,
                ins=[wuin.opt()], outs=[wuout.opt()],
                replica_groups=[list(range(N_CORES))],
            )

            # Activations: two sets of KT tiles [128, BS], ping-pong.
            acts = [
                [apool.tile([128, BS], bf16, name=f"act{s}_{k}")
                 for k in range(KT)]
                for s in range(2)
            ]
            # DRAM bounce buffers for the chunked weight AllGathers.
            # Chunk q of layer l covers rows [q*rs, (q+1)*rs) of each
            # core's 256-row slice; gathered chunk holds 8*rs rows.
            agins = [
                [dram.tile([IS // CHUNKS[l], D], bf16, name=f"agin{l}_{q}")
                 for q in range(CHUNKS[l])]
                for l in range(L)
            ]
            agouts = [
                [dram.tile([N_CORES * (IS // CHUNKS[l]), D], bf16,
                           addr_space="Shared", name=f"agout{l}_{q}")
                 for q in range(CHUNKS[l])]
                for l in range(L)
            ]

            def emit_blend(l):
                C = CHUNKS[l]
                rs = IS // C          # chunk rows per core
                cph = C // 2          # chunks per 128-row half
                for h in range(IS // 128):  # 2 half-slices of 128 partitions
                    acc = bpool.tile([128, D], f32, name=f"acc{l}_{h}",
                                     tag=f"acc{h}", bufs=2)
                    accq = bpool.tile([128, D], bf16, name=f"accq{l}_{h}",
                                      tag=f"accq{h}", bufs=2)
                    for e in range(E):
                        et = epool.tile([128, D], bf16, name=f"exp{l}_{h}_{e}",
                                        tag="exp")
                        nc.scalar.dma_start(
                            et[:], WtT[l, e, h * 128:(h + 1) * 128, :]
                        )
                        if e == 0:
                            nc.vector.tensor_scalar_mul(
                                acc[:], et[:], cbt[:, 0:1]
                            )
                        elif e < E - 1:
                            nc.vector.scalar_tensor_tensor(
                                acc[:], et[:], cbt[:, e:e + 1], acc[:],
                                mybir.AluOpType.mult, mybir.AluOpType.add,
                            )
                        else:
                            nc.vector.scalar_tensor_tensor(
                                accq[:], et[:], cbt[:, e:e + 1], acc[:],
                                mybir.AluOpType.mult, mybir.AluOpType.add,
                            )
                    for qq in range(cph):
                        q = h * cph + qq
                        nc.gpsimd.dma_start(
                            agins[l][q][:], accq[qq * rs:(qq + 1) * rs, :]
                        )
                        nc.gpsimd.collective_compute(
                            "AllGather", mybir.AluOpType.bypass,
                            ins=[agins[l][q].opt()],
                            outs=[agouts[l][q].opt()],
                            replica_groups=[list(range(N_CORES))],
                        )

            emit_blend(0)
            emit_blend(1)

            for k in range(KT):
                nc.sync.dma_start(acts[0][k][:], xT[k * 128:(k + 1) * 128, :])

            def load_ws(l, k, half, name):
                """Stationary tile [128, 1024]: k-tile k of w_l.T, columns
                for m-half `half`, assembled from the gathered chunks."""
                C = CHUNKS[l]
                rs = IS // C
                ws = wpool.tile([128, HALF_M * 128], bf16, name=name,
                                tag="ws")
                s, h = k // 2, k % 2
                cph = C // 2
                cs = half * HALF_M * 128
                for qq in range(128 // rs):   # chunk pieces in this k-tile
                    q = h * cph + qq
                    nc.sync.dma_start(
                        ws[qq * rs:(qq + 1) * rs, :],
                        agouts[l][q][s * rs:(s + 1) * rs,
                                     cs:cs + HALF_M * 128],
                    )
                return ws

            for l in range(L):
                if l == 1:
                    emit_blend(2)
                src = acts[l % 2]
                dst = acts[(l + 1) % 2]
                for half in range(2):
                    psums = [
                        ppool.tile([128, BS], f32,
                                   name=f"ps{l}_{half}_{m}", tag=f"bank{m}")
                        for m in range(HALF_M)
                    ]
                    for k in range(KT):
                        ws = load_ws(l, k, half, f"ws{l}_{half}_{k}")
                        for m in range(HALF_M):
                            nc.tensor.matmul(
                                psums[m][:],
                                ws[:, m * 128:(m + 1) * 128],
                                src[k][:],
                                start=(k == 0),
                                stop=(k == KT - 1),
                            )
                    for m in range(HALF_M):
                        gm = half * HALF_M + m
                        bias = bt[:, l * MT + gm: l * MT + gm + 1]
                        ps = psums[m]
                        if l < L - 1:
                            tt = xpool.tile([128, BS], f32,
                                            name=f"t{l}_{gm}", tag="tmin")
                            zt = xpool.tile([128, BS], f32,
                                            name=f"z{l}_{gm}", tag="zbias")
                            ut = xpool.tile([128, BS], f32,
                                            name=f"u{l}_{gm}", tag="uexp")
                            # t = min(psum + bias, 0) on DVE
                            nc.vector.tensor_scalar(
                                tt[:], ps[:], bias, 0.0,
                                mybir.AluOpType.add, mybir.AluOpType.min,
                            )
                            # z = psum + bias on ACT
                            nc.scalar.activation(
                                zt[:], ps[:],
                                mybir.ActivationFunctionType.Identity,
                                bias=bias,
                            )
                            # u = exp(t) on ACT
                            nc.scalar.activation(
                                ut[:], tt[:], mybir.ActivationFunctionType.Exp
                            )
                            # act_next = max(u - 1, z) on DVE
                            nc.vector.scalar_tensor_tensor(
                                dst[gm][:], ut[:], 1.0, zt[:],
                                mybir.AluOpType.subtract, mybir.AluOpType.max,
                            )
                        else:
                            ot = xpool.tile([128, BS], f32,
                                            name=f"o{gm}", tag="outt")
                            nc.scalar.activation(
                                ot[:], ps[:],
                                mybir.ActivationFunctionType.Identity,
                                bias=bias,
                            )
                            nc.gpsimd.dma_start(
                                yT[gm * 128:(gm + 1) * 128, :], ot[:]
                            )
    nc.finalize()
    return nc


def _get_nc():
    if "nc" not in _cache:
        _cache["nc"] = _build()
    return _cache["nc"]


def make_in_maps(weight_blend, x, W, B):
    weight_blend = np.asarray(weight_blend, dtype=np.float32)
    x = np.asarray(x, dtype=np.float32)
    W = np.asarray(W, dtype=np.float32)
    B = np.asarray(B, dtype=np.float32)

    cb = np.ascontiguousarray(np.broadcast_to(weight_blend[None, :], (128, E)))
    # Bp[p, l*MT+m] = B[l, m*128+p]
    Bp = np.ascontiguousarray(
        B.reshape(L, MT, 128).transpose(2, 0, 1).reshape(128, L * MT)
    )

    import ml_dtypes
    in_maps = []
    for k in range(N_CORES):
        WtTk = np.ascontiguousarray(
            W[:, :, :, k * IS:(k + 1) * IS].transpose(0, 1, 3, 2)
        ).astype(ml_dtypes.bfloat16)
        xTk = np.ascontiguousarray(
            x[k * BS:(k + 1) * BS, :].T).astype(ml_dtypes.bfloat16)
        in_maps.append({"WtT": WtTk, "xT": xTk, "cb": cb, "Bp": Bp})
    return in_maps


def kernel(weight_blend, x, W, B) -> np.ndarray:
    in_maps = make_in_maps(weight_blend, x, W, B)
    nc = _get_nc()
    last_err = None
    for attempt in range(3):
        try:
            res = run_bass_kernel_spmd(nc, in_maps,
                                       core_ids=list(range(N_CORES)))
            out = np.empty((BATCH, D), dtype=np.float32)
            for k in range(N_CORES):
                out[k * BS:(k + 1) * BS, :] = res.results[k]["yT"].T
            if np.isfinite(out).all():
                return out
            last_err = RuntimeError("non-finite kernel output")
        except Exception as e:  # transient NRT/device wedge: retry
            last_err = e
        import time as _time
        _time.sleep(10 * (attempt + 1))
    raise last_err
